# revision 12
# baseline (speedup 1.0000x reference)
"""Trainium2 Bass kernel for the EM-root-spike GLM neuron model.

Strategy (8 NeuronCores, shard time axis with host-side halo — no collectives):
  - Host: concat [S_e | S_i] -> (T, 512) bf16; shard rows [t0-256, t0+25088)
    per core (halo covers the 200-tap causal convs and block padding).
  - Stage 1 (PE): syn = S_cat @ W  via DMA-transposed S tiles as the matmul
    stationary operand -> psum (128 time-in-block, 40 ch) per 128-block,
    cast-copied to bf16 x buffer laid out (d=128, [block, ch]).
  - Stage 2 (PE): the 4 grouped causal convs + 2 spike-history convs as
    block-Toeplitz matmuls: stationary = x channel views (d, block-window),
    moving = host-built Toeplitz matrices of the FIR kernels (d, 2*128).
    Output lands in (block, time-in-block) layout in PSUM.
  - Stage 3 (DVE/ACT): leaf-to-root tree recursion (tanh), root sigmoid/V.
  - DMA out (2, 25088) f32 per core; host trims/concats; tiny out_filters
    computed on host.
"""

import numpy as np
import ml_dtypes

import concourse.bass as bass
import concourse.bacc as bacc
import concourse.mybir as mybir
from concourse.tile import TileContext
from concourse.bass_utils import run_bass_kernel_spmd

BF16 = ml_dtypes.bfloat16
F32 = np.float32

# ---- problem constants (hardcoded per task rules) ----
SUB_NO = 20
T_NO = 200
T_DATA = 200000
E_NO = 400
I_NO = 100
N_BASIS = 20
NCORES = 8

T_LOC = T_DATA // NCORES          # 25000 graded outputs per core
NB_OUT = 196                      # output blocks of 128 -> 25088 computed
T_OUT = NB_OUT * 128              # 25088
PRE_BLK = 2                       # halo blocks prepended (256 samples)
NB_IN = NB_OUT + PRE_BLK          # 198 input blocks
T_IN = NB_IN * 128                # 25344 input rows per core
NCH = 40                          # 20 excitatory + 20 inhibitory channels
KC = 512                          # padded contraction dim (400 e + 100 i + 12 zero)
Z_ROWS = 208                      # 208*128 = 26624 >= T_IN, mult of 16
CHUNKS = ((0, 128), (128, 68))    # (block offset, count) — 196 blocks total

# S window tiling for DMA-transpose loads
WIN_ROWS = 1024                   # rows per window (mult of 16 and 128)
WINS = [(w * WIN_ROWS, WIN_ROWS) for w in range(24)] + [(24 * WIN_ROWS, 768)]

A_COLS = SUB_NO * 2 * 3 * 256 + 3 * 256   # 30720 + 768 = 31488

TRACE = False
LAST = {}


def _toeplitz(kern: np.ndarray, m: int) -> np.ndarray:
    """A[d, i] = kern[i - d + 128*m] (0 outside [0, T_NO))  -> (128, 128)."""
    d = np.arange(128, dtype=np.int64)[:, None]
    i = np.arange(128, dtype=np.int64)[None, :]
    tap = i - d + 128 * m
    valid = (tap >= 0) & (tap < T_NO)
    return np.where(valid, kern[np.clip(tap, 0, T_NO - 1)], 0.0).astype(F32)


def _host_params(inputs):
    """Everything derived from the small parameter tensors, in numpy f32."""
    p = {k: np.asarray(v, dtype=F32) if np.asarray(v).dtype != np.int32 else np.asarray(v)
         for k, v in inputs.items()}
    i_idx = np.arange(N_BASIS, dtype=F32)[:, None]
    phi = (np.pi / 2.0) * i_idx
    x = np.arange(T_NO, dtype=F32)[None, :]
    raw = (5.0 * np.log(x + 1.0)).astype(F32)
    basis = 0.5 * np.cos(raw - phi) + 0.5
    keep = (raw >= phi - np.pi) & (raw <= phi + np.pi)
    cos_basis = np.where(keep, basis, 0.0).astype(F32)

    Tau = p["Tau_syn_ns"]; Delta = p["Delta_syn_ns"]; Wsn = p["W_syn_ns"]
    t = np.arange(T_NO, dtype=F32)[None, :]
    t_e = np.maximum(t - Delta[:, 0:1], 0.0) / (Tau[:, 0:1] ** 2)
    t_i = np.maximum(t - Delta[:, 1:2], 0.0) / (Tau[:, 1:2] ** 2)
    e_kern_ns = (t_e * np.exp(-t_e) * Wsn[:, 0:1] ** 2).astype(F32)
    i_kern_ns = (-t_i * np.exp(-t_i) * Wsn[:, 1:2] ** 2).astype(F32)
    e_kern_s = ((p["W_syn_s"][:, :, 0] ** 2) @ cos_basis).astype(F32)
    i_kern_s = (-(p["W_syn_s"][:, :, 1] ** 2) @ cos_basis).astype(F32)
    hist_s_kern = (p["hist_s_weights"] @ cos_basis).astype(F32)
    hist_ns_kern = (p["hist_ns_weights"] @ cos_basis).astype(F32)

    out_filters = np.vstack(
        [e_kern_ns, i_kern_ns, e_kern_s, i_kern_s,
         hist_ns_kern[None, :], hist_s_kern[None, :]]).astype(F32)

    # W (512, 40): cols 0..19 -> syn_e, 20..39 -> syn_i
    W = np.zeros((KC, NCH), dtype=F32)
    W[:E_NO, :SUB_NO] = p["C_syn_e"].T
    W[E_NO:E_NO + I_NO, SUB_NO:] = p["C_syn_i"].T

    # Toeplitz bank
    A = np.zeros((128, A_COLS), dtype=F32)
    kns = [e_kern_ns, i_kern_ns]
    ks = [e_kern_s, i_kern_s]
    for s in range(SUB_NO):
        for part in range(2):
            for m in range(3):
                base = ((s * 2 + part) * 3 + m) * 256
                A[:, base:base + 128] = _toeplitz(kns[part][s], m)
                A[:, base + 128:base + 256] = _toeplitz(ks[part][s], m)
    for m in range(3):
        base = SUB_NO * 2 * 3 * 256 + m * 256
        A[:, base:base + 128] = _toeplitz(hist_ns_kern, m)
        A[:, base + 128:base + 256] = _toeplitz(hist_s_kern, m)

    w_ns2 = (p["W_sub_ns"] ** 2).astype(F32)
    w_s2 = (p["W_sub_s"] ** 2).astype(F32)
    return dict(
        W=W, A=A, out_filters=out_filters,
        w_ns2=w_ns2, w_s2=w_s2,
        th_ns=p["Theta_ns"], th_s=p["Theta_s"], V_o=float(p["V_o"][0]),
    )


def _build_nc(hp):
    nc = bacc.Bacc("TRN2", target_bir_lowering=False, debug=False,
                   num_devices=NCORES)
    dt = mybir.dt
    S_d = nc.declare_dram_parameter("S", [T_IN, KC], dt.bfloat16, isOutput=False)
    Z_d = nc.declare_dram_parameter("Zr", [Z_ROWS, 128], dt.bfloat16, isOutput=False)
    W_d = nc.declare_dram_parameter("W", [KC, NCH], dt.bfloat16, isOutput=False)
    A_d = nc.declare_dram_parameter("A", [128, A_COLS], dt.bfloat16, isOutput=False)
    out_d = {(tr, ck): nc.declare_dram_parameter(
        f"o{tr}{ck}", [CHUNKS[ck][1], 128], dt.float32, isOutput=True)
        for tr in range(2) for ck in range(2)}

    w_ns2 = hp["w_ns2"]; w_s2 = hp["w_s2"]
    th_ns = hp["th_ns"]; th_s = hp["th_s"]; V_o = hp["V_o"]
    Tanh = mybir.ActivationFunctionType.Tanh
    Sigm = mybir.ActivationFunctionType.Sigmoid
    MUL = mybir.AluOpType.mult
    ADD = mybir.AluOpType.add

    with TileContext(nc) as tc:
        with (
            tc.tile_pool(name="const", bufs=1) as cpool,
            tc.tile_pool(name="sT", bufs=4) as spool,
            tc.tile_pool(name="xbuf", bufs=1) as xpool,
            tc.tile_pool(name="co", bufs=1) as copool,
            tc.tile_pool(name="work", bufs=3) as wpool,
            tc.tile_pool(name="obp", bufs=2) as obpool,
            tc.tile_pool(name="ps_syn", bufs=3, space="PSUM") as ps1,
            tc.tile_pool(name="ps_conv", bufs=4, space="PSUM") as ps2,
        ):
            # ---- constants ----
            W_sb = cpool.tile([128, 4 * NCH], dt.bfloat16, tag="W")
            for c in range(4):
                nc.sync.dma_start(out=W_sb[:, c * NCH:(c + 1) * NCH],
                                  in_=W_d[c * 128:(c + 1) * 128, :])
            A_sb = cpool.tile([128, A_COLS], dt.bfloat16, tag="A")
            nc.sync.dma_start(out=A_sb[:], in_=A_d[:])
            xZ = cpool.tile([128, Z_ROWS], dt.bfloat16, tag="xZ")
            nc.scalar.dma_start(out=xZ[:], in_=Z_d[:], transpose=True)

            # ---- x buffer: (128, [gb, ch]) bf16 ----
            x_sb = xpool.tile([128, NB_IN * NCH], dt.bfloat16, tag="x")
            x_r = x_sb.rearrange("p (b c) -> p b c", c=NCH)
            # co: conv outputs (128, [chunk, s, tree, i]) f32
            co = copool.tile([128, 2 * SUB_NO * 2 * 128], dt.bfloat16, tag="co")
            co_r = co.rearrange("p (c s t i) -> p c s t i", c=2, s=SUB_NO, t=2)
            coh = copool.tile([128, 2 * 2 * 128], dt.float32, tag="coh")
            coh_r = coh.rearrange("p (c t i) -> p c t i", c=2, t=2)

            # ---- stage 1 emission helper ----
            def stage1_window(w):
                r0, rows = WINS[w]
                tiles = []
                for c in range(4):
                    st = spool.tile([128, rows], dt.bfloat16, tag=f"sT{c}")
                    nc.scalar.dma_start(out=st[:],
                                        in_=S_d[r0:r0 + rows, c * 128:(c + 1) * 128],
                                        transpose=True)
                    tiles.append(st)
                for bw in range(rows // 128):
                    gb = r0 // 128 + bw
                    ps = ps1.tile([128, NCH], dt.float32, tag="syn")
                    for c in range(4):
                        nc.tensor.matmul(
                            ps[:, :],
                            lhsT=tiles[c][:, bw * 128:(bw + 1) * 128],
                            rhs=W_sb[:, c * NCH:(c + 1) * NCH],
                            start=(c == 0), stop=(c == 3))
                    nc.vector.tensor_copy(x_r[:, gb, :], ps[:, :])

            # ---- Toeplitz conv emission: one chunk, one subunit ----
            def toeplitz(chunk, s):
                off, cnt = CHUNKS[chunk]
                ps = ps2.tile([128, 256], dt.float32, tag="conv")
                k = 0
                for part in range(2):
                    ch = s + part * SUB_NO
                    for m in range(3):
                        g0 = off + PRE_BLK - m
                        nc.tensor.matmul(
                            ps[0:cnt, :],
                            lhsT=x_r[:, g0:g0 + cnt, ch],
                            rhs=A_sb[:, ((s * 2 + part) * 3 + m) * 256:
                                     ((s * 2 + part) * 3 + m) * 256 + 256],
                            start=(k == 0), stop=(k == 5))
                        k += 1
                nc.vector.tensor_copy(co_r[0:cnt, chunk, s, :, :], ps[0:cnt, :])

            def hist(chunk):
                off, cnt = CHUNKS[chunk]
                ps = ps2.tile([128, 256], dt.float32, tag="conv")
                hbase = SUB_NO * 2 * 3 * 256
                for m in range(3):
                    g0 = off + PRE_BLK - m
                    nc.tensor.matmul(
                        ps[0:cnt, :],
                        lhsT=xZ[:, g0:g0 + cnt],
                        rhs=A_sb[:, hbase + m * 256: hbase + m * 256 + 256],
                        start=(m == 0), stop=(m == 2))
                nc.vector.tensor_copy(coh_r[0:cnt, chunk, :, :], ps[0:cnt, :])

            # ---- tree recursion for one chunk & tree ----
            def recursion(chunk, tree):
                off, cnt = CHUNKS[chunk]
                th = th_ns if tree == 0 else th_s
                w2 = w_ns2 if tree == 0 else w_s2
                ob = obpool.tile([128, SUB_NO * 128], dt.float32, tag="ob")
                ob_r = ob.rearrange("p (s i) -> p s i", s=SUB_NO)

                def cocol(s):
                    return co_r[0:cnt, chunk, s, tree, :]

                for s in range(SUB_NO - 1, 0, -1):
                    c1, c2 = 2 * s + 1, 2 * s + 2
                    if c1 >= SUB_NO:  # leaf
                        nc.scalar.activation(ob_r[0:cnt, s, :], cocol(s), Tanh,
                                             bias=float(th[s]))
                        continue
                    t1 = wpool.tile([128, 128], dt.float32, tag="t1")
                    nc.vector.scalar_tensor_tensor(
                        t1[0:cnt, :], ob_r[0:cnt, c1, :], float(w2[c1]),
                        cocol(s), MUL, ADD)
                    if c2 < SUB_NO:
                        nc.vector.scalar_tensor_tensor(
                            t1[0:cnt, :], ob_r[0:cnt, c2, :], float(w2[c2]),
                            t1[0:cnt, :], MUL, ADD)
                    nc.scalar.activation(ob_r[0:cnt, s, :], t1[0:cnt, :], Tanh,
                                         bias=float(th[s]))
                # root
                t1 = wpool.tile([128, 128], dt.float32, tag="t1")
                nc.vector.scalar_tensor_tensor(
                    t1[0:cnt, :], ob_r[0:cnt, 1, :], float(w2[1]), cocol(0),
                    MUL, ADD)
                nc.vector.scalar_tensor_tensor(
                    t1[0:cnt, :], ob_r[0:cnt, 2, :], float(w2[2]), t1[0:cnt, :],
                    MUL, ADD)
                nc.vector.tensor_add(t1[0:cnt, :], t1[0:cnt, :],
                                     coh_r[0:cnt, chunk, tree, :])
                if tree == 0:
                    t2 = wpool.tile([128, 128], dt.float32, tag="t1")
                    nc.scalar.activation(t2[0:cnt, :], t1[0:cnt, :], Tanh,
                                         bias=float(th[0]))
                    fin = wpool.tile([128, 128], dt.float32, tag="fin")
                    nc.vector.tensor_scalar(fin[0:cnt, :], t2[0:cnt, :],
                                            float(w_ns2[0]), V_o, MUL, ADD)
                else:
                    fin = wpool.tile([128, 128], dt.float32, tag="fin")
                    nc.scalar.activation(fin[0:cnt, :], t1[0:cnt, :], Sigm,
                                         bias=float(th[0]))
                nc.sync.dma_start(out=out_d[(tree, chunk)][:],
                                  in_=fin[0:cnt, :])

            # ---- emission order (chunk0 compute overlaps chunk1 loads) ----
            for w in range(17):          # blocks 0..135 (chunk0 needs <=129)
                stage1_window(w)
            for s in range(SUB_NO):
                toeplitz(0, s)
            hist(0)
            for w in range(17, len(WINS)):
                stage1_window(w)
            for tree in range(2):
                recursion(0, tree)
            for s in range(SUB_NO):
                toeplitz(1, s)
            hist(1)
            for tree in range(2):
                recursion(1, tree)
    nc.finalize()
    return nc


def kernel(**inputs):
    hp = _host_params(inputs)
    S_e = np.asarray(inputs["S_e"], dtype=F32)
    S_i = np.asarray(inputs["S_i"], dtype=F32)
    Z = np.asarray(inputs["Z"], dtype=F32)

    S_cat = np.zeros((T_DATA, KC), dtype=BF16)
    S_cat[:, :E_NO] = S_e.astype(BF16)
    S_cat[:, E_NO:E_NO + I_NO] = S_i.astype(BF16)

    in_maps = []
    for k in range(NCORES):
        t0 = k * T_LOC
        lo, hi = t0 - PRE_BLK * 128, t0 + T_OUT
        Ssh = np.zeros((T_IN, KC), dtype=BF16)
        a, b = max(lo, 0), min(hi, T_DATA)
        Ssh[a - lo:b - lo] = S_cat[a:b]
        Zsh = np.zeros((Z_ROWS * 128,), dtype=BF16)
        zlo = t0 - PRE_BLK * 128 - 1          # Zsh[k] = Z[zlo + k]
        a, b = max(zlo, 0), min(zlo + Z_ROWS * 128, T_DATA)
        Zsh[a - zlo:b - zlo] = Z[a:b].astype(BF16)
        in_maps.append({
            "S": Ssh,
            "Zr": Zsh.reshape(Z_ROWS, 128),
            "W": hp["W"].astype(BF16),
            "A": hp["A"].astype(BF16),
        })

    nc = _build_nc(hp)
    res = run_bass_kernel_spmd(nc, in_maps, core_ids=list(range(NCORES)),
                               trace=TRACE)
    LAST["exec_time_ns"] = res.exec_time_ns
    LAST["results"] = res
    LAST["nc"] = nc
    LAST["in_maps"] = in_maps

    def _gather(tr):
        outs = []
        for k in range(NCORES):
            r = res.results[k]
            full = np.concatenate([np.asarray(r[f"o{tr}0"]).reshape(-1),
                                   np.asarray(r[f"o{tr}1"]).reshape(-1)])
            outs.append(full[:T_LOC])
        return np.concatenate(outs)
    V = _gather(0)
    Zout = _gather(1)
    return (V.astype(F32), Zout.astype(F32), hp["out_filters"])


# ---------------- benchmarking (test-only; not used by grading) ----------------

def _make_sharded_fn(nc, in_maps):
    import jax
    from jax.sharding import Mesh, PartitionSpec
    from jax.experimental.shard_map import shard_map
    from concourse import bass2jax
    from concourse import mybir as _mb
    bass2jax.install_neuronx_cc_hook()

    pname = nc.partition_id_tensor.name if nc.partition_id_tensor else None
    in_names, out_names, out_avals = [], [], []
    for alloc in nc.m.functions[0].allocations:
        if not isinstance(alloc, _mb.MemoryLocationSet):
            continue
        name = alloc.memorylocations[0].name
        if alloc.kind == "ExternalInput":
            if name != pname:
                in_names.append(name)
        elif alloc.kind == "ExternalOutput":
            out_names.append(name)
            out_avals.append(jax.core.ShapedArray(
                tuple(alloc.tensor_shape), _mb.dt.np(alloc.dtype)))
    n_params = len(in_names)
    all_names = in_names + out_names
    if pname is not None:
        all_names = all_names + [pname]

    def _body(*args):
        operands = list(args)
        if pname is not None:
            operands.append(bass2jax.partition_id_tensor())
        return tuple(bass2jax._bass_exec_p.bind(
            *operands, out_avals=tuple(out_avals), in_names=tuple(all_names),
            out_names=tuple(out_names), lowering_input_output_aliases=(),
            sim_require_finite=True, sim_require_nnan=True, nc=nc))

    n = len(in_maps)
    mesh = Mesh(np.asarray(jax.devices()[:n]), ("core",))
    specs = (PartitionSpec("core"),) * (n_params + len(out_names))
    fn = jax.jit(shard_map(_body, mesh=mesh, in_specs=specs,
                           out_specs=(PartitionSpec("core"),) * len(out_names),
                           check_rep=False), keep_unused=True)
    args = [np.concatenate([np.asarray(m[name]) for m in in_maps], axis=0)
            for name in in_names]
    args += [np.zeros((n * a.shape[0], *a.shape[1:]), a.dtype) for a in out_avals]
    dev_args = [jax.device_put(a) for a in args]
    return fn, dev_args


def bench(iters=30, nc=None, in_maps=None):
    import time as _time
    import jax
    nc = nc or LAST["nc"]
    in_maps = in_maps or LAST["in_maps"]
    fn, dev_args = _make_sharded_fn(nc, in_maps)
    out = fn(*dev_args); jax.block_until_ready(out)
    t0 = _time.perf_counter()
    for _ in range(iters):
        out = fn(*dev_args)
    jax.block_until_ready(out)
    per = (_time.perf_counter() - t0) / iters
    return per * 1e9


def bench_null(iters=30):
    """Dispatch-overhead floor: trivial copy NEFF on all 8 cores."""
    nc = bacc.Bacc("TRN2", target_bir_lowering=False, debug=False,
                   num_devices=NCORES)
    dt = mybir.dt
    a = nc.declare_dram_parameter("a", [128, 128], dt.float32, isOutput=False)
    o = nc.declare_dram_parameter("o", [128, 128], dt.float32, isOutput=True)
    with TileContext(nc) as tc:
        with tc.tile_pool(name="p", bufs=1) as pool:
            t = pool.tile([128, 128], dt.float32)
            nc.sync.dma_start(out=t[:], in_=a[:])
            nc.sync.dma_start(out=o[:], in_=t[:])
    nc.finalize()
    maps = [{"a": np.zeros((128, 128), F32)} for _ in range(NCORES)]
    return bench(iters, nc=nc, in_maps=maps)
